# revision 90
# baseline (speedup 1.0000x reference)
"""Trainium2 Bass kernel for the MetricLearning pairwise loss.

Reference math:
    d2[i,j] = max(||x_i||^2 + ||x_j||^2 - 2 x_i.x_j, EPS)
    a = d2/(2k)/sigma^2 ; b = d2/(2k)/omega^2 ; c1 = k/2-1
    per_pair = same ? (-c1*log(a) + a/2) : (c1*log(b) - b/2)
    loss = sum_{i<j} per_pair

Split: everything linear in d2 has a closed form the host computes exactly
in fp64; the device computes only the two log sums
    S1 = sum_{i<j} ln(d2),   S2 = sum_{same,i<j} ln(d2).

Device pipeline: fp8 DoubleRow matmul chains produce  t = -d2/2  directly
in PSUM -- BOTH norm terms ride inside the contraction as aug features:
features 1020/1021 carry -sq_i/2 on the lhs (rhs side const 4.0), features
1022/1023 carry -sq_j/2 on the rhs (lhs side const 4.0), each as an fp8
hi/lo split at weight 4.  Work is cut into ~10 pair-groups, each filling
a 2-bank [P,1024] PSUM tile with 1-3 matmul chains; ONE Ln activation
per group (scale=-2, no bias) drops the plain sums straight into a
[P,32] accumulator via accum_out (columns with equal coefficients share
an accumulator), the masked/halved sums go through chunked DVE
add-reduces; the accumulator is DMA'd out raw and the host does the
final 128-way partition sum in fp64.

Rows are globally SORTED BY LABEL (max run <= 128), so same-label pairs
live only inside a 256-block or in the 128-wide corner between
consecutive blocks.  Block diagonals avoid full-tile redundancy: unit u0
computes its block's [128,256] (triangle via the symmetric trick + the
in-block cross counted once), unit u1 computes only its own [128,128]
triangle (clamped diagonal, halved sum).

Sharding: 16 row-blocks of 256; K8 super-node orientation gives every
core 10 resident blocks (identical SPMD program, per-core slab
permutation).

Schedule notes (hard-won against the traces):
 - DMA completion semaphores drain behind ALL queued data packets, so
   arrival order == issue order; slab0's halves open BOTH HW DGE queues,
   aux (lhsx+rowlab packed as one u8 tensor) rides sync behind them, and
   slabs 1-9 stream whole on sync.  The scalar queue carries only tiny
   transfers: its engine (ACT) must be free for the Ln stream (big DMA
   issues on it starve PSUM recycling and stall the PE).
 - A 12-matmul PE warmup burst (weights memset by the otherwise-idle
   DVE) bridges the entry barrier to slab0's arrival, and 3 more filler
   matmuls sit at the two known DMA-arrival gaps (slab1-wait before C,
   slab2-wait after the deferred B chain), so the HAM activity window
   stays hot and the 2.4 GHz unthrottle fires at ~11us; the real chains
   then stream at the fp8-DoubleRow issue rate (~216ns per 512-wide
   matmul, measured total excess over that baseline: ~0.15us).
 - Pairs are ordered to match slab arrival; P1/P2 share one PSUM tile
   and P3 absorbs E2, keeping the 4-tile pool rotation ahead of the PE.
 - The accumulator leaves in two DMAs (bulk after P9, tail columns
   after the trailing S+T Ln + accumulator drain); the serial tail
   after the last matmul is ~1.1us plus fixed DMA-ring/exit latency.
"""

import numpy as np
import ml_dtypes

N = 4096
D = 1024
P = 128
NB = 16          # row blocks
BLK = 256        # rows per block
KC = D // P      # k chunks (8)
NCORES = 8
NSLOT = 10       # distinct blocks resident per core
NSH = NSLOT * 2  # half-slab count

SIGMA = 0.2
OMEGA = 1.0
K_F = float(N)
C1 = K_F / 2.0 - 1.0                      # 2047
A_C = 1.0 / (2.0 * K_F * SIGMA * SIGMA)
B_C = 1.0 / (2.0 * K_F * OMEGA * OMEGA)
LOG_A = float(np.log(A_C))
LOG_B = float(np.log(B_C))
MARGIN = 128.0   # diag clamp floor; raw diag |d2| < ~50, off-diag > ~1400
LNM = float(np.log(MARGIN))
ACC_W = 32

# K8 super-node orientation: core c owns 3 super-edges (first one is
# c+1 so the consecutive-block corner lands at slot 2), plus one
# crosswise-split super-pair.  Covers all 120 block pairs exactly once.
OWNED = {0: [1, 7, 6], 1: [2, 6, 7], 2: [3, 4, 5], 3: [4, 0, 5],
         4: [5, 0, 1], 5: [6, 0, 1], 6: [7, 3, 2], 7: [2, 3, 4]}
MATCH = {0: 2, 2: 0, 1: 3, 3: 1, 4: 6, 6: 4, 5: 7, 7: 5}

# acc-column schema (mirrors the device emission order):
#   'full'   S1 += v          'half'   S1 += (v - 128*lnM)/2
#   'm_full' S2 += v          'm_half' S2 += (v - 128*lnM)/2
_SCHEMA = (
    ['half', 'full', 'half', 'm_half', 'm_full', 'm_half']   # P1: A1+B
    + ['half', 'full', 'half', 'm_half', 'm_full', 'm_half']  # P2: C+D
    + ['full', 'm_full']                                      # P3: A2+E1
    + ['full', 'full', 'm_full']                              # P4: E2+F
    + ['full'] * 5                                            # P5-P9
    + ['full']                                                # P10: Q+R
    + ['full']                                                # P11: S+T
)
assert len(_SCHEMA) <= ACC_W, len(_SCHEMA)

NWARM = 11       # PE warmup matmuls before real chains; sized so the
                 # warmup burst (~256ns each, cold) ends right when
                 # slab0 has landed (~10.3us) -- the HAM activity window
                 # then sees no gap and unthrottles to 2.4 GHz at ~11us,
                 # so the real chains run mostly warm


def _core_slabs(d):
    slabs = [2 * d, 2 * d + 1]
    for o in OWNED[d]:
        slabs += [2 * o, 2 * o + 1]
    cp = MATCH[d]
    if d < cp:
        slabs += [2 * cp, 2 * cp + 1]
    else:
        slabs += [2 * cp + 1, 2 * cp]
    assert len(slabs) == NSLOT and len(set(slabs)) == NSLOT
    return slabs


_PROG_CACHE = {}


def _build_program():
    if "nc" in _PROG_CACHE:
        return _PROG_CACHE["nc"]
    import concourse.bass as bass  # noqa: F401
    import concourse.bacc as bacc
    import concourse.mybir as mybir
    import concourse.tile as tile

    F32 = mybir.dt.float32
    BF16 = mybir.dt.bfloat16
    FP8 = mybir.dt.float8e4
    AF = mybir.ActivationFunctionType
    ALU = mybir.AluOpType
    DR = mybir.MatmulPerfMode.DoubleRow
    AX = mybir.AxisListType.X

    U8 = mybir.dt.uint8
    nc = bacc.Bacc("TRN2", target_bir_lowering=False, debug=False,
                   num_devices=NCORES)
    xtp_d = nc.dram_tensor("xtp", [NSLOT, P, 2, KC * P], FP8,
                           kind="ExternalInput").ap()
    aux_d = nc.dram_tensor("aux", [P, 1040], U8, kind="ExternalInput").ap()
    lab_d = nc.dram_tensor("lab", [1, 640], BF16, kind="ExternalInput").ap()
    out_d = nc.dram_tensor("out", [P, ACC_W], F32, kind="ExternalOutput").ap()

    with tile.TileContext(nc) as tc:
        with (
            tc.tile_pool(name="persist", bufs=1) as persist,
            tc.tile_pool(name="ltpool", bufs=4) as ltpool,
            tc.tile_pool(name="tcpool", bufs=2) as tcpool,
            tc.tile_pool(name="psum", bufs=4, space="PSUM") as psum,
        ):
            xall = persist.tile([P, NSH, KC, P], FP8, tag="xall")
            aux = persist.tile([P, 1040], U8, tag="aux")
            lhsx = aux[:, 0:1024].bitcast(FP8).rearrange(
                "p (g k m) -> p g k m", g=4, k=2)
            rl = aux[:, 1024:1040].bitcast(F32)
            labb = persist.tile([P, 640], F32, tag="labb")
            labr = persist.tile([1, 640], BF16, tag="labr")
            ones2 = persist.tile([2, P], BF16, tag="ones2")
            acc = persist.tile([P, ACC_W], F32, tag="acc")
            maskb = persist.tile([P, 1024], F32, tag="maskb")
            prodb = persist.tile([P, 1024], F32, tag="prodb")
            wm8 = persist.tile([P, 512], FP8, tag="wm8")
            wsink = persist.tile([P, 1], F32, tag="wsink")

            # DMA: completion semaphores drain behind all queued data
            # packets (global FIFO across the 16 engines), so arrival
            # order == issue order and early inputs must be issued with
            # little data ahead of them.  slab0 halves open both queues;
            # lhsx for units 0/1 rides right behind so the first kp=3
            # passes don't stall; slabs 1-9 stream whole on sync, leaving
            # the scalar queue free for the Ln stream.
            nc.sync.dma_start(out=xall[:, 0], in_=xtp_d[0, :, 0])
            nc.scalar.dma_start(out=xall[:, 1], in_=xtp_d[0, :, 1])
            nc.sync.dma_start(out=aux[:], in_=aux_d[:])
            nc.scalar.dma_start(out=labr[:], in_=lab_d[:])
            for s in range(1, NSLOT):
                nc.sync.dma_start(out=xall[:, 2 * s:2 * s + 2],
                                  in_=xtp_d[s])

            # wm8 memset on the otherwise-idle DVE so the first warmup
            # LDW is released as early as possible after the entry barrier
            nc.vector.memset(wm8[:], 1.0)
            nc.gpsimd.memset(ones2[:], 1.0)
            nc.gpsimd.memset(acc[:], 0.0)

            # PE warmup: DoubleRow dummies keep the HAM activity window
            # busy from the entry barrier until slab0 lands, so the 2.4
            # GHz unthrottle fires right after the real chains start
            wlhs = wm8[:, 0:256].rearrange("p (k m) -> p k m", k=2)
            wrhs = wm8[:].rearrange("p (k c) -> p k c", k=2)
            wt = psum.tile([P, 1024], F32, tag="seg")
            for i in range(NWARM):
                nc.tensor.matmul(wt[:, 0:256], wlhs, wrhs,
                                 start=True, stop=True, perf_mode=DR)
            nc.vector.tensor_copy(wsink[:], wt[:, 0:1])

            def mm_chain(t_ap, g, sh0, nsh):
                ls, u = g >> 1, g & 1
                for kp in range(KC // 2):
                    if kp == KC // 2 - 1:
                        lhs = lhsx[:, g, :, :]
                    else:
                        lhs = xall[:, 2 * ls + u, 2 * kp:2 * kp + 2, :]
                    nc.tensor.matmul(
                        t_ap, lhs,
                        xall[:, sh0:sh0 + nsh, 2 * kp:2 * kp + 2, :]
                            .rearrange("p s k c -> p k s c"),
                        start=(kp == 0), stop=(kp == KC // 2 - 1),
                        perf_mode=DR)

            col = [0]
            moff = [0]

            def next_cols(n):
                c = col[0]
                col[0] += n
                return c

            def creduce(src_ap, nchunk, chunk):
                c = next_cols(nchunk)
                nc.vector.tensor_reduce(
                    acc[:, c:c + nchunk],
                    src_ap.rearrange("p (a b) -> p a b", a=nchunk),
                    axis=AX, op=ALU.add)

            def masked(lt_ap, mo, mw, lab_off, g):
                mk = maskb[:, mo:mo + mw]
                pr = prodb[:, mo:mo + mw]
                nc.vector.tensor_scalar(mk, labb[:, lab_off:lab_off + mw],
                                        rl[:, g:g + 1], None, ALU.is_equal)
                nc.vector.tensor_tensor(pr, mk, lt_ap, ALU.mult)
                return pr

            # --- P1 (A1: u0 x slot0 spec, B: u1 x slot0h1 tri) and
            # --- P2 (C: u2 x slot1 spec, D: u3 x slot1h1 tri) share one
            # 2-bank tile (384 cols each, bank-aligned halves); the WAR
            # coupling lands inside the DMA-gated early window
            tg1 = psum.tile([P, 1024], F32, tag="seg")
            # A1 split into halves: the tri half needs only slab0-h0 (the
            # first DMA to land), so the PE starts before sh1 arrives.
            # B (u1 tri, slab0-only) is deferred into P3 below to fill
            # the PE gap when slab2 arrives late.
            mm_chain(tg1[:, 0:128], 0, 0, 1)
            mm_chain(tg1[:, 128:256], 0, 1, 1)
            # label-row broadcast rides the PE between the first chains
            # (the DVE copies out of PSUM are deferred past the mins so a
            # late labr arrival can't head-block the DVE queue)
            pls = []
            for lo, w in ((0, 512), (512, 128)):
                pl = psum.tile([P, 1024], F32, tag="seg")
                nc.tensor.matmul(pl[:, 0:w], ones2[0:1, :],
                                 labr[0:1, lo:lo + w],
                                 start=True, stop=True)
                pls.append(pl)
            # two fillers bridge the slab1-arrival gap in slow-DMA runs
            # (C is data-gated past this point anyway, so they are ~free
            # in fast runs but keep the HAM activity window alive)
            for i in range(2):
                nc.tensor.matmul(pls[1][:, 512:768], wlhs, wrhs,
                                 start=True, stop=True, perf_mode=DR)
            mm_chain(tg1[:, 512:768], 2, 2, 2)
            mm_chain(tg1[:, 768:896], 3, 3, 1)
            # --- P3: A2 (u0 x slot1) + E1 (u1 x slot1, corner A in its
            # first 128) -- the whole pair needs only slab1, packing more
            # PE work into the earliest DMA arrivals
            tg = psum.tile([P, 1024], F32, tag="seg")
            mm_chain(tg[:, 0:256], 0, 2, 2)      # A2
            mm_chain(tg[:, 256:512], 1, 2, 2)    # E1
            mm_chain(tg1[:, 256:384], 1, 1, 1)   # B (slab0-only gap filler)
            # one more filler bridges to slab2's arrival
            nc.tensor.matmul(tg[:, 768:1024], wlhs, wrhs,
                             start=True, stop=True, perf_mode=DR)
            mm_chain(tg[:, 512:768], 1, 4, 2)    # E2 (u1 x slot2)
            # both mins first (they are tg1's only readers -- freeing the
            # shared tile for the pool is on the PE critical path), masks
            # and reductions after
            lts = []
            for ofs in (0, 512):
                tcb = tcpool.tile([P, 384], F32, tag="tc")
                nc.vector.tensor_scalar(tcb[:], tg1[:, ofs:ofs + 384],
                                        -MARGIN / 2.0, None, ALU.min)
                lt = ltpool.tile([P, 1024], F32, tag="lt")
                nc.scalar.activation(lt[:, 0:384], tcb[:], AF.Ln, scale=-2.0)
                lts.append(lt)
            for pli, (lo, w) in zip(pls, ((0, 512), (512, 128))):
                nc.vector.tensor_copy(labb[:, lo:lo + w], pli[:, 0:w])
            for lt, ga, gb, lab_off in ((lts[0], 0, 1, 0),
                                        (lts[1], 2, 3, 256)):
                creduce(lt[:, 0:384], 3, 128)
                mo = moff[0]
                moff[0] += 384
                mk = maskb[:, mo:mo + 384]
                pr = prodb[:, mo:mo + 384]
                nc.vector.tensor_scalar(mk[:, 0:256],
                                        labb[:, lab_off:lab_off + 256],
                                        rl[:, ga:ga + 1], None, ALU.is_equal)
                nc.vector.tensor_scalar(mk[:, 256:384],
                                        labb[:, lab_off + 128:lab_off + 256],
                                        rl[:, gb:gb + 1], None, ALU.is_equal)
                nc.vector.tensor_tensor(pr, mk, lt[:, 0:384], ALU.mult)
                creduce(pr, 3, 128)

            lt = ltpool.tile([P, 1024], F32, tag="lt")
            c = next_cols(1)
            nc.scalar.activation(lt[:, 0:512], tg[:, 0:512], AF.Ln,
                                 scale=-2.0, accum_out=acc[:, c:c + 1])
            pr = masked(lt[:, 256:384], moff[0], 128, 256, 1)
            moff[0] += 128
            creduce(pr, 1, 128)
            c = next_cols(1)
            nc.scalar.activation(lt[:, 512:768], tg[:, 512:768], AF.Ln,
                                 scale=-2.0, accum_out=acc[:, c:c + 1])

            # --- P4: F (u3 x slots2-3, corner B in cols 0:128)
            tg = psum.tile([P, 1024], F32, tag="seg")
            mm_chain(tg[:, 0:512], 3, 4, 4)      # F
            lt = ltpool.tile([P, 1024], F32, tag="lt")
            c = next_cols(1)
            nc.scalar.activation(lt[:, 0:512], tg[:, 0:512], AF.Ln,
                                 scale=-2.0, accum_out=acc[:, c:c + 1])
            pr = masked(lt[:, 0:128], moff[0], 128, 512, 3)
            moff[0] += 128
            creduce(pr, 1, 128)

            # --- P5-P9: plain 512+512 pairs; one Ln + accumulator per
            # pair (both halves share an S1 'full' column)
            for (g1, sha, g2, shb) in ((0, 4, 2, 4), (1, 6, 0, 8),
                                       (2, 8, 3, 8), (1, 10, 0, 12),
                                       (2, 12, 3, 12)):
                tg = psum.tile([P, 1024], F32, tag="seg")
                mm_chain(tg[:, 0:512], g1, sha, 4)
                mm_chain(tg[:, 512:1024], g2, shb, 4)
                lt = ltpool.tile([P, 1024], F32, tag="lt")
                c = next_cols(1)
                nc.scalar.activation(lt[:], tg[:], AF.Ln, scale=-2.0,
                                     accum_out=acc[:, c:c + 1])

            # --- P10: Q (u1 x slots7-8) + R (u0 x slot8)
            tg = psum.tile([P, 1024], F32, tag="seg")
            mm_chain(tg[:, 0:512], 1, 14, 4)     # Q
            mm_chain(tg[:, 512:768], 0, 16, 2)   # R
            lt = ltpool.tile([P, 1024], F32, tag="lt")
            c = next_cols(1)
            nc.scalar.activation(lt[:, 0:768], tg[:, 0:768], AF.Ln,
                                 scale=-2.0, accum_out=acc[:, c:c + 1])

            # --- P11: S + T (u2/u3 x slot9) share one tile and one Ln,
            # so the serial tail after the last matmul is a single
            # 512-wide Ln plus its accumulator drain
            tg = psum.tile([P, 1024], F32, tag="seg")
            mm_chain(tg[:, 0:256], 2, 18, 2)     # S
            mm_chain(tg[:, 256:512], 3, 18, 2)   # T
            lt = ltpool.tile([P, 1024], F32, tag="lt")
            c = next_cols(1)
            nc.scalar.activation(lt[:, 0:512], tg[:, 0:512], AF.Ln,
                                 scale=-2.0, accum_out=acc[:, c:c + 1])

            assert col[0] == len(_SCHEMA), col[0]
            assert moff[0] == 1024, moff[0]
            # ship the accumulator in two pieces: the bulk leaves as soon
            # as P1-P9's columns are final, only the last 10 columns wait
            # for the trailing pairs' accumulator drains
            nc.sync.dma_start(out=out_d[:, 0:22], in_=acc[:, 0:22])
            nc.sync.dma_start(out=out_d[:, 22:ACC_W], in_=acc[:, 22:ACC_W])

    nc.compile()
    _PROG_CACHE["nc"] = nc
    return nc


def _host_prep(outputs, labels):
    """Sort rows by label, build per-core inputs + exact linear terms."""
    x = np.asarray(outputs, dtype=np.float32)
    lab = np.asarray(labels)
    assert x.shape == (N, D)
    perm = np.argsort(lab, kind="stable")
    xp = x[perm]
    labp = lab[perm].astype(np.float64)

    runs_end = np.empty(N, dtype=np.int64)
    i = 0
    max_run = 0
    while i < N:
        j = i
        while j < N and labp[j] == labp[i]:
            j += 1
        runs_end[i:j] = j
        max_run = max(max_run, j - i)
        i = j
    assert max_run <= P, f"label run {max_run} exceeds corner width"

    xq = xp.astype(ml_dtypes.float8_e4m3)
    # True (unquantized) norms make d2 = sq_i + sq_j - 2*xq_i.xq_j unbiased:
    # the value-error correlation in ||xq||^2 cancels the ||e||^2 term.
    x64 = xp.astype(np.float64)
    sq = (x64 ** 2).sum(axis=1)

    # exact linear terms (fp64 closed form, true values)
    npairs = N * (N - 1) // 2
    ssum = x64.sum(axis=0)
    d2_all = N * sq.sum() - float(ssum @ ssum)
    nsame = 0
    d2_same = 0.0
    i = 0
    while i < N:
        j = int(runs_end[i])
        ng = j - i
        nsame += ng * (ng - 1) // 2
        sg = x64[i:j].sum(axis=0)
        d2_same += ng * sq[i:j].sum() - float(sg @ sg)
        i = j
    host_const = (C1 * npairs * LOG_B - (B_C / 2.0) * d2_all
                  - C1 * (LOG_A + LOG_B) * nsame
                  + ((A_C + B_C) / 2.0) * d2_same)

    # fp8 hi/lo split of -sq/2 at weight 4.0 (e4m3 max 448, -sq/8 ~ -128)
    r0 = (-sq / 8.0).astype(ml_dtypes.float8_e4m3)
    r1 = ((-sq / 2.0 - 4.0 * r0.astype(np.float64)) / 4.0).astype(
        ml_dtypes.float8_e4m3)
    sqq = -8.0 * (r0.astype(np.float64) + r1.astype(np.float64))

    # rhs view: features 1020/1021 const 4.0 (lhs-aug partners), 1022/1023
    # carry -sq_j hi/lo
    xq_rhs = xq.copy()
    xq_rhs[:, 1020] = 4.0
    xq_rhs[:, 1021] = 4.0
    xq_rhs[:, 1022] = r0
    xq_rhs[:, 1023] = r1

    # device diag: d2 = 2*sqq - 2*sum_{f<1020} xq^2 must clamp inside MARGIN
    sq8p = (xq[:, :1020].astype(np.float64) ** 2).sum(axis=1)
    d2diag = 2.0 * sqq - 2.0 * sq8p
    assert np.abs(d2diag).max() < MARGIN - 16, np.abs(d2diag).max()

    xt_q = np.ascontiguousarray(xq_rhs.T)                           # [D, N]

    in_maps = []
    for d in range(NCORES):
        slabs = _core_slabs(d)
        # xtp[s, p, h, k*128+c] = xq_rhs[blockrow(s) + h*128 + c, k*128 + p]
        cols = np.concatenate(
            [np.arange(b * BLK, (b + 1) * BLK) for b in slabs])
        xtp = np.ascontiguousarray(
            xt_q[:, cols].reshape(KC, P, NSLOT, 2, P)
            .transpose(2, 1, 3, 0, 4).reshape(NSLOT, P, 2, KC * P))
        # lhs tensor for the LAST k-pair (chunks 6-7): quantized x features,
        # rows 1020/1021 (chunk 7, partitions 124/125) hold -sq_i hi/lo,
        # rows 1022/1023 (partitions 126/127) hold the aug weight 4.0
        lhsxa = np.empty((P, 4, 2, P), dtype=ml_dtypes.float8_e4m3)
        rowlab = np.zeros((P, 4), dtype=np.float32)
        for g, (slab, u) in enumerate(((0, 0), (0, 1), (1, 0), (1, 1))):
            rows = slabs[slab] * BLK + 128 * u + np.arange(P)
            blk = xq[rows, (KC - 2) * P:].reshape(P, 2, P)
            lhsxa[:, g] = blk.transpose(2, 1, 0)    # [part, chunk, row m]
            lhsxa[124, g, 1, :] = r0[rows]
            lhsxa[125, g, 1, :] = r1[rows]
            rowlab[:, g] = labp[rows]
        lhsxa[126, :, 1, :] = 4.0
        lhsxa[127, :, 1, :] = 4.0
        # label row for slot0(256) | slot1(256) | slot2 first 128
        cols0 = np.concatenate(
            [np.arange(b * BLK, (b + 1) * BLK) for b in slabs[:2]]
            + [np.arange(slabs[2] * BLK, slabs[2] * BLK + 128)])
        labrow = labp[cols0].astype(ml_dtypes.bfloat16)[None, :]   # [1, 640]

        aux = np.concatenate(
            [np.ascontiguousarray(lhsxa).reshape(P, 1024).view(np.uint8),
             np.ascontiguousarray(rowlab).view(np.uint8)], axis=1)
        in_maps.append({
            "xtp": xtp,
            "aux": np.ascontiguousarray(aux),
            "lab": np.ascontiguousarray(labrow),
        })
    return in_maps, host_const


def _finalize(host_const, outs_list):
    """Combine per-core raw accumulators [P, ACC_W] with the closed form."""
    total = np.float64(host_const)
    s1 = 0.0
    s2 = 0.0
    for o in outs_list:
        v = np.asarray(o, dtype=np.float64).reshape(P, ACC_W).sum(axis=0)
        for c, kindc in enumerate(_SCHEMA):
            if kindc == 'full':
                s1 += v[c]
            elif kindc == 'half':
                s1 += (v[c] - P * LNM) / 2.0
            elif kindc == 'm_full':
                s2 += v[c]
            else:
                s2 += (v[c] - P * LNM) / 2.0
    total += C1 * s1 - 2.0 * C1 * s2
    return np.asarray(total, dtype=np.float32)


def kernel(**inputs):
    from concourse.bass_utils import run_bass_kernel_spmd
    nc = _build_program()
    in_maps, host_const = _host_prep(inputs["outputs"], inputs["labels"])
    res = run_bass_kernel_spmd(nc, in_maps, core_ids=list(range(NCORES)))
    return _finalize(host_const, [r["out"] for r in res.results])


# revision 91
# speedup vs baseline: 1.0887x; 1.0887x over previous
"""Trainium2 Bass kernel for the MetricLearning pairwise loss.

Reference math:
    d2[i,j] = max(||x_i||^2 + ||x_j||^2 - 2 x_i.x_j, EPS)
    a = d2/(2k)/sigma^2 ; b = d2/(2k)/omega^2 ; c1 = k/2-1
    per_pair = same ? (-c1*log(a) + a/2) : (c1*log(b) - b/2)
    loss = sum_{i<j} per_pair

Split: everything linear in d2 has a closed form the host computes exactly
in fp64; the device computes only the two log sums
    S1 = sum_{i<j} ln(d2),   S2 = sum_{same,i<j} ln(d2).

Device pipeline: fp8 DoubleRow matmul chains produce  t = -d2/2  directly
in PSUM -- BOTH norm terms ride inside the contraction as aug features:
features 1020/1021 carry -sq_i/2 on the lhs (rhs side const 4.0), features
1022/1023 carry -sq_j/2 on the rhs (lhs side const 4.0), each as an fp8
hi/lo split at weight 4.  Work is cut into ~10 pair-groups, each filling
a 2-bank [P,1024] PSUM tile with 1-3 matmul chains; ONE Ln activation
per group (scale=-2, no bias) drops the plain sums straight into a
[P,32] accumulator via accum_out (columns with equal coefficients share
an accumulator), the masked/halved sums go through chunked DVE
add-reduces; the accumulator is DMA'd out raw and the host does the
final 128-way partition sum in fp64.

Rows are globally SORTED BY LABEL (max run <= 128), so same-label pairs
live only inside a 256-block or in the 128-wide corner between
consecutive blocks.  Block diagonals avoid full-tile redundancy: unit u0
computes its block's [128,256] (triangle via the symmetric trick + the
in-block cross counted once), unit u1 computes only its own [128,128]
triangle (clamped diagonal, halved sum).

Sharding: 16 row-blocks of 256; K8 super-node orientation gives every
core 10 resident blocks (identical SPMD program, per-core slab
permutation).

Schedule notes (hard-won against the traces):
 - DMA completion semaphores drain behind ALL queued data packets, so
   arrival order == issue order; slab0's halves open BOTH HW DGE queues,
   aux (lhsx+rowlab packed as one u8 tensor) rides sync behind them, and
   slabs 1-9 stream whole on sync.  The scalar queue carries only tiny
   transfers: its engine (ACT) must be free for the Ln stream (big DMA
   issues on it starve PSUM recycling and stall the PE).
 - A 12-matmul PE warmup burst (weights memset by the otherwise-idle
   DVE) bridges the entry barrier to slab0's arrival, and 3 more filler
   matmuls sit at the two known DMA-arrival gaps (slab1-wait before C,
   slab2-wait after the deferred B chain), so the HAM activity window
   stays hot and the 2.4 GHz unthrottle fires at ~11us; the real chains
   then stream at the fp8-DoubleRow issue rate (~216ns per 512-wide
   matmul, measured total excess over that baseline: ~0.15us).
 - Pairs are ordered to match slab arrival; P1/P2 share one PSUM tile
   and P3 absorbs E2, keeping the 4-tile pool rotation ahead of the PE.
 - The accumulator leaves in two DMAs (bulk after P9, tail columns
   after the trailing S+T Ln + accumulator drain); the serial tail
   after the last matmul is ~1.1us plus fixed DMA-ring/exit latency.
"""

import numpy as np
import ml_dtypes

N = 4096
D = 1024
P = 128
NB = 16          # row blocks
BLK = 256        # rows per block
KC = D // P      # k chunks (8)
NCORES = 8
NSLOT = 10       # distinct blocks resident per core
NSH = NSLOT * 2  # half-slab count

SIGMA = 0.2
OMEGA = 1.0
K_F = float(N)
C1 = K_F / 2.0 - 1.0                      # 2047
A_C = 1.0 / (2.0 * K_F * SIGMA * SIGMA)
B_C = 1.0 / (2.0 * K_F * OMEGA * OMEGA)
LOG_A = float(np.log(A_C))
LOG_B = float(np.log(B_C))
MARGIN = 128.0   # diag clamp floor; raw diag |d2| < ~50, off-diag > ~1400
LNM = float(np.log(MARGIN))
ACC_W = 32

# K8 super-node orientation: core c owns 3 super-edges (first one is
# c+1 so the consecutive-block corner lands at slot 2), plus one
# crosswise-split super-pair.  Covers all 120 block pairs exactly once.
OWNED = {0: [1, 7, 6], 1: [2, 6, 7], 2: [3, 4, 5], 3: [4, 0, 5],
         4: [5, 0, 1], 5: [6, 0, 1], 6: [7, 3, 2], 7: [2, 3, 4]}
MATCH = {0: 2, 2: 0, 1: 3, 3: 1, 4: 6, 6: 4, 5: 7, 7: 5}

# acc-column schema (mirrors the device emission order):
#   'full'   S1 += v          'half'   S1 += (v - 128*lnM)/2
#   'm_full' S2 += v          'm_half' S2 += (v - 128*lnM)/2
_SCHEMA = (
    ['half', 'full', 'half', 'm_half', 'm_full', 'm_half']   # P1: A1+B
    + ['half', 'full', 'half', 'm_half', 'm_full', 'm_half']  # P2: C+D
    + ['full', 'm_full']                                      # P3: A2+E1
    + ['full', 'full', 'm_full']                              # P4: E2+F
    + ['full'] * 5                                            # P5-P9
    + ['full']                                                # P10: Q+R
    + ['full']                                                # P11: S+T
)
assert len(_SCHEMA) <= ACC_W, len(_SCHEMA)

NWARM = 12       # PE warmup matmuls before real chains; sized so the
                 # warmup burst (~256ns each, cold) ends right when
                 # slab0 has landed (~10.3us) -- the HAM activity window
                 # then sees no gap and unthrottles to 2.4 GHz at ~11us,
                 # so the real chains run mostly warm


def _core_slabs(d):
    slabs = [2 * d, 2 * d + 1]
    for o in OWNED[d]:
        slabs += [2 * o, 2 * o + 1]
    cp = MATCH[d]
    if d < cp:
        slabs += [2 * cp, 2 * cp + 1]
    else:
        slabs += [2 * cp + 1, 2 * cp]
    assert len(slabs) == NSLOT and len(set(slabs)) == NSLOT
    return slabs


_PROG_CACHE = {}


def _build_program():
    if "nc" in _PROG_CACHE:
        return _PROG_CACHE["nc"]
    import concourse.bass as bass  # noqa: F401
    import concourse.bacc as bacc
    import concourse.mybir as mybir
    import concourse.tile as tile

    F32 = mybir.dt.float32
    BF16 = mybir.dt.bfloat16
    FP8 = mybir.dt.float8e4
    AF = mybir.ActivationFunctionType
    ALU = mybir.AluOpType
    DR = mybir.MatmulPerfMode.DoubleRow
    AX = mybir.AxisListType.X

    U8 = mybir.dt.uint8
    nc = bacc.Bacc("TRN2", target_bir_lowering=False, debug=False,
                   num_devices=NCORES)
    xtp_d = nc.dram_tensor("xtp", [NSLOT, P, 2, KC * P], FP8,
                           kind="ExternalInput").ap()
    aux_d = nc.dram_tensor("aux", [P, 1040], U8, kind="ExternalInput").ap()
    lab_d = nc.dram_tensor("lab", [1, 640], BF16, kind="ExternalInput").ap()
    out_d = nc.dram_tensor("out", [P, ACC_W], F32, kind="ExternalOutput").ap()

    with tile.TileContext(nc) as tc:
        with (
            tc.tile_pool(name="persist", bufs=1) as persist,
            tc.tile_pool(name="ltpool", bufs=4) as ltpool,
            tc.tile_pool(name="tcpool", bufs=2) as tcpool,
            tc.tile_pool(name="psum", bufs=4, space="PSUM") as psum,
        ):
            xall = persist.tile([P, NSH, KC, P], FP8, tag="xall")
            aux = persist.tile([P, 1040], U8, tag="aux")
            lhsx = aux[:, 0:1024].bitcast(FP8).rearrange(
                "p (g k m) -> p g k m", g=4, k=2)
            rl = aux[:, 1024:1040].bitcast(F32)
            labb = persist.tile([P, 640], F32, tag="labb")
            labr = persist.tile([1, 640], BF16, tag="labr")
            ones2 = persist.tile([2, P], BF16, tag="ones2")
            acc = persist.tile([P, ACC_W], F32, tag="acc")
            maskb = persist.tile([P, 1024], F32, tag="maskb")
            prodb = persist.tile([P, 1024], F32, tag="prodb")
            wm8 = persist.tile([P, 512], FP8, tag="wm8")
            wsink = persist.tile([P, 1], F32, tag="wsink")

            # DMA: completion semaphores drain behind all queued data
            # packets (global FIFO across the 16 engines), so arrival
            # order == issue order and early inputs must be issued with
            # little data ahead of them.  slab0 halves open both queues;
            # lhsx for units 0/1 rides right behind so the first kp=3
            # passes don't stall; slabs 1-9 stream whole on sync, leaving
            # the scalar queue free for the Ln stream.
            nc.sync.dma_start(out=xall[:, 0], in_=xtp_d[0, :, 0])
            nc.scalar.dma_start(out=xall[:, 1], in_=xtp_d[0, :, 1])
            nc.sync.dma_start(out=aux[:], in_=aux_d[:])
            nc.scalar.dma_start(out=labr[:], in_=lab_d[:])
            for s in range(1, NSLOT):
                nc.sync.dma_start(out=xall[:, 2 * s:2 * s + 2],
                                  in_=xtp_d[s])

            # wm8 memset on the otherwise-idle DVE so the first warmup
            # LDW is released as early as possible after the entry barrier
            nc.vector.memset(wm8[:], 1.0)
            nc.gpsimd.memset(ones2[:], 1.0)
            nc.gpsimd.memset(acc[:], 0.0)

            # PE warmup: DoubleRow dummies keep the HAM activity window
            # busy from the entry barrier until slab0 lands, so the 2.4
            # GHz unthrottle fires right after the real chains start
            wlhs = wm8[:, 0:256].rearrange("p (k m) -> p k m", k=2)
            wrhs = wm8[:].rearrange("p (k c) -> p k c", k=2)
            wt = psum.tile([P, 1024], F32, tag="seg")
            for i in range(NWARM):
                nc.tensor.matmul(wt[:, 0:256], wlhs, wrhs,
                                 start=True, stop=True, perf_mode=DR)
            nc.vector.tensor_copy(wsink[:], wt[:, 0:1])

            def mm_chain(t_ap, g, sh0, nsh):
                ls, u = g >> 1, g & 1
                for kp in range(KC // 2):
                    if kp == KC // 2 - 1:
                        lhs = lhsx[:, g, :, :]
                    else:
                        lhs = xall[:, 2 * ls + u, 2 * kp:2 * kp + 2, :]
                    nc.tensor.matmul(
                        t_ap, lhs,
                        xall[:, sh0:sh0 + nsh, 2 * kp:2 * kp + 2, :]
                            .rearrange("p s k c -> p k s c"),
                        start=(kp == 0), stop=(kp == KC // 2 - 1),
                        perf_mode=DR)

            col = [0]
            moff = [0]

            def next_cols(n):
                c = col[0]
                col[0] += n
                return c

            def creduce(src_ap, nchunk, chunk):
                c = next_cols(nchunk)
                nc.vector.tensor_reduce(
                    acc[:, c:c + nchunk],
                    src_ap.rearrange("p (a b) -> p a b", a=nchunk),
                    axis=AX, op=ALU.add)

            def masked(lt_ap, mo, mw, lab_off, g):
                mk = maskb[:, mo:mo + mw]
                pr = prodb[:, mo:mo + mw]
                nc.vector.tensor_scalar(mk, labb[:, lab_off:lab_off + mw],
                                        rl[:, g:g + 1], None, ALU.is_equal)
                nc.vector.tensor_tensor(pr, mk, lt_ap, ALU.mult)
                return pr

            # --- P1 (A1: u0 x slot0 spec, B: u1 x slot0h1 tri) and
            # --- P2 (C: u2 x slot1 spec, D: u3 x slot1h1 tri) share one
            # 2-bank tile (384 cols each, bank-aligned halves); the WAR
            # coupling lands inside the DMA-gated early window
            tg1 = psum.tile([P, 1024], F32, tag="seg")
            # A1 split into halves: the tri half needs only slab0-h0 (the
            # first DMA to land), so the PE starts before sh1 arrives.
            # B (u1 tri, slab0-only) is deferred into P3 below to fill
            # the PE gap when slab2 arrives late.
            mm_chain(tg1[:, 0:128], 0, 0, 1)
            mm_chain(tg1[:, 128:256], 0, 1, 1)
            # label-row broadcast rides the PE between the first chains
            # (the DVE copies out of PSUM are deferred past the mins so a
            # late labr arrival can't head-block the DVE queue)
            pls = []
            for lo, w in ((0, 512), (512, 128)):
                pl = psum.tile([P, 1024], F32, tag="seg")
                nc.tensor.matmul(pl[:, 0:w], ones2[0:1, :],
                                 labr[0:1, lo:lo + w],
                                 start=True, stop=True)
                pls.append(pl)
            # two fillers bridge the slab1-arrival gap in slow-DMA runs
            # (C is data-gated past this point anyway, so they are ~free
            # in fast runs but keep the HAM activity window alive)
            for i in range(2):
                nc.tensor.matmul(pls[1][:, 512:768], wlhs, wrhs,
                                 start=True, stop=True, perf_mode=DR)
            mm_chain(tg1[:, 512:768], 2, 2, 2)
            mm_chain(tg1[:, 768:896], 3, 3, 1)
            # --- P3: A2 (u0 x slot1) + E1 (u1 x slot1, corner A in its
            # first 128) -- the whole pair needs only slab1, packing more
            # PE work into the earliest DMA arrivals
            tg = psum.tile([P, 1024], F32, tag="seg")
            mm_chain(tg[:, 0:256], 0, 2, 2)      # A2
            mm_chain(tg[:, 256:512], 1, 2, 2)    # E1
            mm_chain(tg1[:, 256:384], 1, 1, 1)   # B (slab0-only gap filler)
            # one more filler bridges to slab2's arrival
            nc.tensor.matmul(tg[:, 768:1024], wlhs, wrhs,
                             start=True, stop=True, perf_mode=DR)
            mm_chain(tg[:, 512:768], 1, 4, 2)    # E2 (u1 x slot2)
            # both mins first (they are tg1's only readers -- freeing the
            # shared tile for the pool is on the PE critical path), masks
            # and reductions after
            lts = []
            for ofs in (0, 512):
                tcb = tcpool.tile([P, 384], F32, tag="tc")
                nc.vector.tensor_scalar(tcb[:], tg1[:, ofs:ofs + 384],
                                        -MARGIN / 2.0, None, ALU.min)
                lt = ltpool.tile([P, 1024], F32, tag="lt")
                nc.scalar.activation(lt[:, 0:384], tcb[:], AF.Ln, scale=-2.0)
                lts.append(lt)
            for pli, (lo, w) in zip(pls, ((0, 512), (512, 128))):
                nc.vector.tensor_copy(labb[:, lo:lo + w], pli[:, 0:w])
            for lt, ga, gb, lab_off in ((lts[0], 0, 1, 0),
                                        (lts[1], 2, 3, 256)):
                creduce(lt[:, 0:384], 3, 128)
                mo = moff[0]
                moff[0] += 384
                mk = maskb[:, mo:mo + 384]
                pr = prodb[:, mo:mo + 384]
                nc.vector.tensor_scalar(mk[:, 0:256],
                                        labb[:, lab_off:lab_off + 256],
                                        rl[:, ga:ga + 1], None, ALU.is_equal)
                nc.vector.tensor_scalar(mk[:, 256:384],
                                        labb[:, lab_off + 128:lab_off + 256],
                                        rl[:, gb:gb + 1], None, ALU.is_equal)
                nc.vector.tensor_tensor(pr, mk, lt[:, 0:384], ALU.mult)
                creduce(pr, 3, 128)

            lt = ltpool.tile([P, 1024], F32, tag="lt")
            c = next_cols(1)
            nc.scalar.activation(lt[:, 0:512], tg[:, 0:512], AF.Ln,
                                 scale=-2.0, accum_out=acc[:, c:c + 1])
            pr = masked(lt[:, 256:384], moff[0], 128, 256, 1)
            moff[0] += 128
            creduce(pr, 1, 128)
            c = next_cols(1)
            nc.scalar.activation(lt[:, 512:768], tg[:, 512:768], AF.Ln,
                                 scale=-2.0, accum_out=acc[:, c:c + 1])

            # --- P4: F (u3 x slots2-3, corner B in cols 0:128)
            tg = psum.tile([P, 1024], F32, tag="seg")
            mm_chain(tg[:, 0:512], 3, 4, 4)      # F
            lt = ltpool.tile([P, 1024], F32, tag="lt")
            c = next_cols(1)
            nc.scalar.activation(lt[:, 0:512], tg[:, 0:512], AF.Ln,
                                 scale=-2.0, accum_out=acc[:, c:c + 1])
            pr = masked(lt[:, 0:128], moff[0], 128, 512, 3)
            moff[0] += 128
            creduce(pr, 1, 128)

            # --- P5-P9: plain 512+512 pairs; one Ln + accumulator per
            # pair (both halves share an S1 'full' column)
            for (g1, sha, g2, shb) in ((0, 4, 2, 4), (1, 6, 0, 8),
                                       (2, 8, 3, 8), (1, 10, 0, 12),
                                       (2, 12, 3, 12)):
                tg = psum.tile([P, 1024], F32, tag="seg")
                mm_chain(tg[:, 0:512], g1, sha, 4)
                mm_chain(tg[:, 512:1024], g2, shb, 4)
                lt = ltpool.tile([P, 1024], F32, tag="lt")
                c = next_cols(1)
                nc.scalar.activation(lt[:], tg[:], AF.Ln, scale=-2.0,
                                     accum_out=acc[:, c:c + 1])

            # --- P10: Q (u1 x slots7-8) + R (u0 x slot8)
            tg = psum.tile([P, 1024], F32, tag="seg")
            mm_chain(tg[:, 0:512], 1, 14, 4)     # Q
            mm_chain(tg[:, 512:768], 0, 16, 2)   # R
            lt = ltpool.tile([P, 1024], F32, tag="lt")
            c = next_cols(1)
            nc.scalar.activation(lt[:, 0:768], tg[:, 0:768], AF.Ln,
                                 scale=-2.0, accum_out=acc[:, c:c + 1])

            # --- P11: S + T (u2/u3 x slot9) share one tile and one Ln,
            # so the serial tail after the last matmul is a single
            # 512-wide Ln plus its accumulator drain
            tg = psum.tile([P, 1024], F32, tag="seg")
            mm_chain(tg[:, 0:256], 2, 18, 2)     # S
            mm_chain(tg[:, 256:512], 3, 18, 2)   # T
            lt = ltpool.tile([P, 1024], F32, tag="lt")
            c = next_cols(1)
            nc.scalar.activation(lt[:, 0:512], tg[:, 0:512], AF.Ln,
                                 scale=-2.0, accum_out=acc[:, c:c + 1])

            assert col[0] == len(_SCHEMA), col[0]
            assert moff[0] == 1024, moff[0]
            # ship the accumulator in two pieces: the bulk leaves as soon
            # as P1-P9's columns are final, only the last 10 columns wait
            # for the trailing pairs' accumulator drains
            nc.sync.dma_start(out=out_d[:, 0:22], in_=acc[:, 0:22])
            nc.sync.dma_start(out=out_d[:, 22:ACC_W], in_=acc[:, 22:ACC_W])

    nc.compile()
    _PROG_CACHE["nc"] = nc
    return nc


def _host_prep(outputs, labels):
    """Sort rows by label, build per-core inputs + exact linear terms."""
    x = np.asarray(outputs, dtype=np.float32)
    lab = np.asarray(labels)
    assert x.shape == (N, D)
    perm = np.argsort(lab, kind="stable")
    xp = x[perm]
    labp = lab[perm].astype(np.float64)

    runs_end = np.empty(N, dtype=np.int64)
    i = 0
    max_run = 0
    while i < N:
        j = i
        while j < N and labp[j] == labp[i]:
            j += 1
        runs_end[i:j] = j
        max_run = max(max_run, j - i)
        i = j
    assert max_run <= P, f"label run {max_run} exceeds corner width"

    xq = xp.astype(ml_dtypes.float8_e4m3)
    # True (unquantized) norms make d2 = sq_i + sq_j - 2*xq_i.xq_j unbiased:
    # the value-error correlation in ||xq||^2 cancels the ||e||^2 term.
    x64 = xp.astype(np.float64)
    sq = (x64 ** 2).sum(axis=1)

    # exact linear terms (fp64 closed form, true values)
    npairs = N * (N - 1) // 2
    ssum = x64.sum(axis=0)
    d2_all = N * sq.sum() - float(ssum @ ssum)
    nsame = 0
    d2_same = 0.0
    i = 0
    while i < N:
        j = int(runs_end[i])
        ng = j - i
        nsame += ng * (ng - 1) // 2
        sg = x64[i:j].sum(axis=0)
        d2_same += ng * sq[i:j].sum() - float(sg @ sg)
        i = j
    host_const = (C1 * npairs * LOG_B - (B_C / 2.0) * d2_all
                  - C1 * (LOG_A + LOG_B) * nsame
                  + ((A_C + B_C) / 2.0) * d2_same)

    # fp8 hi/lo split of -sq/2 at weight 4.0 (e4m3 max 448, -sq/8 ~ -128)
    r0 = (-sq / 8.0).astype(ml_dtypes.float8_e4m3)
    r1 = ((-sq / 2.0 - 4.0 * r0.astype(np.float64)) / 4.0).astype(
        ml_dtypes.float8_e4m3)
    sqq = -8.0 * (r0.astype(np.float64) + r1.astype(np.float64))

    # rhs view: features 1020/1021 const 4.0 (lhs-aug partners), 1022/1023
    # carry -sq_j hi/lo
    xq_rhs = xq.copy()
    xq_rhs[:, 1020] = 4.0
    xq_rhs[:, 1021] = 4.0
    xq_rhs[:, 1022] = r0
    xq_rhs[:, 1023] = r1

    # device diag: d2 = 2*sqq - 2*sum_{f<1020} xq^2 must clamp inside MARGIN
    sq8p = (xq[:, :1020].astype(np.float64) ** 2).sum(axis=1)
    d2diag = 2.0 * sqq - 2.0 * sq8p
    assert np.abs(d2diag).max() < MARGIN - 16, np.abs(d2diag).max()

    xt_q = np.ascontiguousarray(xq_rhs.T)                           # [D, N]

    in_maps = []
    for d in range(NCORES):
        slabs = _core_slabs(d)
        # xtp[s, p, h, k*128+c] = xq_rhs[blockrow(s) + h*128 + c, k*128 + p]
        cols = np.concatenate(
            [np.arange(b * BLK, (b + 1) * BLK) for b in slabs])
        xtp = np.ascontiguousarray(
            xt_q[:, cols].reshape(KC, P, NSLOT, 2, P)
            .transpose(2, 1, 3, 0, 4).reshape(NSLOT, P, 2, KC * P))
        # lhs tensor for the LAST k-pair (chunks 6-7): quantized x features,
        # rows 1020/1021 (chunk 7, partitions 124/125) hold -sq_i hi/lo,
        # rows 1022/1023 (partitions 126/127) hold the aug weight 4.0
        lhsxa = np.empty((P, 4, 2, P), dtype=ml_dtypes.float8_e4m3)
        rowlab = np.zeros((P, 4), dtype=np.float32)
        for g, (slab, u) in enumerate(((0, 0), (0, 1), (1, 0), (1, 1))):
            rows = slabs[slab] * BLK + 128 * u + np.arange(P)
            blk = xq[rows, (KC - 2) * P:].reshape(P, 2, P)
            lhsxa[:, g] = blk.transpose(2, 1, 0)    # [part, chunk, row m]
            lhsxa[124, g, 1, :] = r0[rows]
            lhsxa[125, g, 1, :] = r1[rows]
            rowlab[:, g] = labp[rows]
        lhsxa[126, :, 1, :] = 4.0
        lhsxa[127, :, 1, :] = 4.0
        # label row for slot0(256) | slot1(256) | slot2 first 128
        cols0 = np.concatenate(
            [np.arange(b * BLK, (b + 1) * BLK) for b in slabs[:2]]
            + [np.arange(slabs[2] * BLK, slabs[2] * BLK + 128)])
        labrow = labp[cols0].astype(ml_dtypes.bfloat16)[None, :]   # [1, 640]

        aux = np.concatenate(
            [np.ascontiguousarray(lhsxa).reshape(P, 1024).view(np.uint8),
             np.ascontiguousarray(rowlab).view(np.uint8)], axis=1)
        in_maps.append({
            "xtp": xtp,
            "aux": np.ascontiguousarray(aux),
            "lab": np.ascontiguousarray(labrow),
        })
    return in_maps, host_const


def _finalize(host_const, outs_list):
    """Combine per-core raw accumulators [P, ACC_W] with the closed form."""
    total = np.float64(host_const)
    s1 = 0.0
    s2 = 0.0
    for o in outs_list:
        v = np.asarray(o, dtype=np.float64).reshape(P, ACC_W).sum(axis=0)
        for c, kindc in enumerate(_SCHEMA):
            if kindc == 'full':
                s1 += v[c]
            elif kindc == 'half':
                s1 += (v[c] - P * LNM) / 2.0
            elif kindc == 'm_full':
                s2 += v[c]
            else:
                s2 += (v[c] - P * LNM) / 2.0
    total += C1 * s1 - 2.0 * C1 * s2
    return np.asarray(total, dtype=np.float32)


def kernel(**inputs):
    from concourse.bass_utils import run_bass_kernel_spmd
    nc = _build_program()
    in_maps, host_const = _host_prep(inputs["outputs"], inputs["labels"])
    res = run_bass_kernel_spmd(nc, in_maps, core_ids=list(range(NCORES)))
    return _finalize(host_const, [r["out"] for r in res.results])
